# revision 6
# baseline (speedup 1.0000x reference)
"""Trainium2 Bass kernel for nn_Attention_28269474742408.

Single-layer attention block: qkv projections -> softmax attention ->
layernorm -> output projection, for x [8, 1024, 768] (B=8, N=1024, C=768,
H=12 heads, D=64).

Strategy: data parallel over the batch — one batch element per NeuronCore
(8 cores). Everything on-chip per core; no collectives.

Per-core layout (all channel-major, "T" = [channel, token]):
  - Host pre-transposes x[b] -> xT [768, 1024] and all weights -> W.T so
    projections/attention never need on-device transposes.
  - qT, kT [768, 1024]: q/k projections computed directly transposed.
  - v kept token-major [1024, 780]: 12 heads x (64 v-cols + a ones column);
    the ones column makes the PV matmul emit softmax denominators for free.
  - scores computed as S.T [m, n] per head; softmax skips max-subtraction
    (scores are bounded ~|3|, exp can't overflow) so exp needs no
    cross-partition reduction; denominators come from the ones column.
  - attnT [64+1, 1024] per head accumulates over m-tiles in PSUM
    (flash-style: no [1024, 1024] score materialization).
  - LayerNorm is folded into the output projection: gamma/beta folded into
    Wo/bo on the host; mean/var via ones-matmuls (cross-partition sums);
    the -mean*colsum(Wo) + sqrt(var+eps)*bo rank-2 correction rides the
    output matmul as an extra K=2 accumulation; the rsqrt scale is applied
    per-token on eviction.
  - All matmuls run in float32r (fp32 storage, ~bf16 speed at N>=256,
    ~1.6e-4 matmul relative error).
"""
import numpy as np

import concourse.bacc as bacc
import concourse.bass as bass
import concourse.tile as tile
from concourse import mybir
from concourse.bass_utils import run_bass_kernel_spmd

F32 = mybir.dt.float32
F32R = mybir.dt.float32r
AF = mybir.ActivationFunctionType
OP = mybir.AluOpType

B, N, C, H, D = 8, 1024, 768, 12, 64
KT = C // 128          # 6 channel tiles
NT = N // 128          # 8 token tiles
VW = H * (D + 1)       # 780: v plus per-head ones column
SCALE = D ** -0.5
EPS = 1e-5

# mega column offsets: [wq | wk | wv_ext | xT]
OFF_WQ, OFF_WK, OFF_WV, OFF_XT = 0, C, 2 * C, 2 * C + VW
MEGA_W = 2 * C + VW + N  # 3340


def build_kernel():
    nc = bacc.Bacc("TRN2", target_bir_lowering=False)

    mega = nc.dram_tensor("mega", (C, MEGA_W), F32R, kind="ExternalInput")
    wo_d = nc.dram_tensor("wo", (C, C), F32R, kind="ExternalInput")
    extra_d = nc.dram_tensor("extra", (2, C), F32R, kind="ExternalInput")
    bqs_d = nc.dram_tensor("bqs", (C,), F32, kind="ExternalInput")
    bve_d = nc.dram_tensor("bve", (VW,), F32R, kind="ExternalInput")
    ones_d = nc.dram_tensor("onesd", (128,), F32R, kind="ExternalInput")
    ones2_d = nc.dram_tensor("ones2", (128, 2), F32R, kind="ExternalInput")
    y_d = nc.dram_tensor("y", (N, C), F32, kind="ExternalOutput")
    rscr = nc.dram_tensor("rscr", (H, N), F32)   # internal: recip bounce

    with tile.TileContext(nc) as tc:
        with tc.tile_pool(name="persist", bufs=1) as pp, \
             tc.tile_pool(name="attp", bufs=1) as attp:

            # ---- constants / small loads ----
            extra_t = pp.tile([2, C], F32R, tag="extra", name="extra")
            nc.sync.dma_start(out=extra_t, in_=extra_d[:, :])
            bqs_t = [pp.tile([128, 1], F32, tag=f"bqs{m}", name=f"bqs{m}") for m in range(KT)]
            for m in range(KT):
                nc.sync.dma_start(out=bqs_t[m], in_=bqs_d[m * 128:(m + 1) * 128].unsqueeze(1))
            bve_t = pp.tile([1, VW], F32R, tag="bve", name="bve")
            nc.sync.dma_start(out=bve_t, in_=bve_d[:].unsqueeze(0))
            ones_row = pp.tile([1, 128], F32R, tag="ones_row", name="ones_row")
            nc.sync.dma_start(out=ones_row, in_=ones_d[:].unsqueeze(0))
            ones_col = pp.tile([128, 1], F32R, tag="ones_col", name="ones_col")
            nc.sync.dma_start(out=ones_col, in_=ones_d[:].unsqueeze(1))
            ones2_t = pp.tile([128, 2], F32R, tag="ones2t", name="ones2t")
            nc.sync.dma_start(out=ones2_t, in_=ones2_d[:, :])
            eps_col = pp.tile([128, 1], F32, tag="eps_col", name="eps_col")
            nc.vector.memset(eps_col, EPS)
            eps_row = pp.tile([1, 1], F32, tag="eps_row", name="eps_row")
            nc.vector.memset(eps_row, EPS)

            att = [attp.tile([128, N], F32R, tag=f"att{k}", name=f"att{k}") for k in range(KT)]
            f_t = pp.tile([2, N], F32R, tag="f_t", name="f_t")        # [-mean; sqrt(var+eps)]
            acol = pp.tile([128, NT], F32, tag="acol", name="acol")    # rsqrt(var+eps) per token

            with tc.tile_pool(name="qk", bufs=1) as qkp, \
                 tc.tile_pool(name="vp", bufs=1) as vp:
                qt = [qkp.tile([128, N], F32R, tag=f"qt{m}", name=f"qt{m}") for m in range(KT)]
                kt = [qkp.tile([128, N], F32R, tag=f"kt{m}", name=f"kt{m}") for m in range(KT)]
                vt = [vp.tile([128, VW], F32R, tag=f"vt{n}", name=f"vt{n}") for n in range(NT)]

                # ---- phase A: projections ----
                with tc.tile_pool(name="megap", bufs=1) as megap, \
                     tc.tile_pool(name="ps_qk", bufs=3, space="PSUM") as ps_qk, \
                     tc.tile_pool(name="ps_v", bufs=2, space="PSUM") as ps_v:
                    mg = [megap.tile([128, MEGA_W], F32R, tag=f"mg{k}", name=f"mg{k}") for k in range(KT)]
                    for k in range(KT):
                        nc.sync.dma_start(out=mg[k], in_=mega[k * 128:(k + 1) * 128, :])

                    # v (token-major, bias + ones column folded into the rhs)
                    for n in range(NT):
                        for c0, cw in ((0, 512), (512, VW - 512)):
                            pv = ps_v.tile([128, 512], F32, tag="pv", name="pv")
                            for k in range(KT):
                                nc.tensor.matmul(
                                    out=pv[:, 0:cw],
                                    lhsT=mg[k][:, OFF_XT + n * 128: OFF_XT + (n + 1) * 128],
                                    rhs=mg[k][:, OFF_WV + c0: OFF_WV + c0 + cw],
                                    start=(k == 0), stop=False,
                                )
                            nc.tensor.matmul(
                                out=pv[:, 0:cw],
                                lhsT=ones_row,
                                rhs=bve_t[:, c0:c0 + cw],
                                start=False, stop=True,
                            )
                            nc.vector.tensor_copy(out=vt[n][:, c0:c0 + cw], in_=pv[:, 0:cw])

                    # qT (scaled + bias) and kT (no bias: softmax-invariant)
                    for m in range(KT):
                        for ch in range(2):
                            pq = ps_qk.tile([128, 512], F32, tag="pq", name="pq")
                            for k in range(KT):
                                nc.tensor.matmul(
                                    out=pq,
                                    lhsT=mg[k][:, OFF_WQ + m * 128: OFF_WQ + (m + 1) * 128],
                                    rhs=mg[k][:, OFF_XT + ch * 512: OFF_XT + (ch + 1) * 512],
                                    start=(k == 0), stop=(k == KT - 1),
                                )
                            nc.vector.tensor_scalar(
                                out=qt[m][:, ch * 512:(ch + 1) * 512], in0=pq,
                                scalar1=bqs_t[m], scalar2=None, op0=OP.add,
                            )
                            pk = ps_qk.tile([128, 512], F32, tag="pq", name="pq")
                            for k in range(KT):
                                nc.tensor.matmul(
                                    out=pk,
                                    lhsT=mg[k][:, OFF_WK + m * 128: OFF_WK + (m + 1) * 128],
                                    rhs=mg[k][:, OFF_XT + ch * 512: OFF_XT + (ch + 1) * 512],
                                    start=(k == 0), stop=(k == KT - 1),
                                )
                            nc.vector.tensor_copy(
                                out=kt[m][:, ch * 512:(ch + 1) * 512], in_=pk)

                # ---- phase B: attention, head by head (flash-style over m) ----
                with tc.tile_pool(name="epool", bufs=3) as epool, \
                     tc.tile_pool(name="aupool", bufs=2) as aupool, \
                     tc.tile_pool(name="drpool", bufs=2) as drpool, \
                     tc.tile_pool(name="rbcpool", bufs=2) as rbcpool, \
                     tc.tile_pool(name="ps_s", bufs=2, space="PSUM") as ps_s, \
                     tc.tile_pool(name="ps_att", bufs=2, space="PSUM") as ps_att:
                    for h in range(H):
                        ht, hr = h // 2, (h % 2) * 64
                        q_h = qt[ht][hr:hr + 64, :]
                        k_h = kt[ht][hr:hr + 64, :]
                        pa = ps_att.tile([65, N], F32, tag="pa", name="pa")
                        for mt in range(NT):
                            sp = ps_s.tile([128, N], F32, tag="sp", name="sp")
                            for ch in range(2):
                                nc.tensor.matmul(
                                    out=sp[:, ch * 512:(ch + 1) * 512],
                                    lhsT=k_h[:, mt * 128:(mt + 1) * 128],
                                    rhs=q_h[:, ch * 512:(ch + 1) * 512],
                                    start=True, stop=True,
                                )
                            e = epool.tile([128, N], F32R, tag="e", name="e")
                            nc.scalar.activation(out=e, in_=sp, func=AF.Exp)
                            for ch in range(2):
                                nc.tensor.matmul(
                                    out=pa[:, ch * 512:(ch + 1) * 512],
                                    lhsT=vt[mt][:, h * 65:(h + 1) * 65],
                                    rhs=e[:, ch * 512:(ch + 1) * 512],
                                    start=(mt == 0), stop=(mt == NT - 1),
                                )
                        # evict numerator + denominator; divide via DRAM-bounced
                        # row broadcast (per-token reciprocal along the free dim)
                        attu = aupool.tile([64, N], F32, tag="attu", name="attu")
                        nc.scalar.copy(out=attu, in_=pa[0:64, :])
                        den = drpool.tile([1, N], F32, tag="den", name="den")
                        nc.scalar.copy(out=den, in_=pa[64:65, :])
                        rec = drpool.tile([1, N], F32, tag="rec", name="rec")
                        nc.vector.reciprocal(out=rec, in_=den)
                        nc.sync.dma_start(out=rscr[h:h + 1, :], in_=rec)
                        rbc = rbcpool.tile([64, N], F32, tag="rbc", name="rbc")
                        src = rscr[h:h + 1, :]
                        nc.sync.dma_start(
                            out=rbc,
                            in_=bass.AP(tensor=src.tensor, offset=src.offset,
                                        ap=[[0, 64]] + [list(d) for d in src.ap[1:]]),
                        )
                        nc.vector.tensor_tensor(
                            out=att[ht][hr:hr + 64, :], in0=attu, in1=rbc, op=OP.mult)

            # ---- phase C: LN stats + output projection ----
            with tc.tile_pool(name="wop", bufs=1) as wop, \
                 tc.tile_pool(name="sqp", bufs=1) as sqp, \
                 tc.tile_pool(name="rowpool", bufs=2) as rowpool, \
                 tc.tile_pool(name="ypool", bufs=2) as ypool:
                wo_t = [wop.tile([128, C], F32R, tag=f"wo{k}", name=f"wo{k}") for k in range(KT)]
                for k in range(KT):
                    nc.sync.dma_start(out=wo_t[k], in_=wo_d[k * 128:(k + 1) * 128, :])

                sq = [sqp.tile([128, N], F32R, tag=f"sq{k}", name=f"sq{k}") for k in range(KT)]
                for k in range(KT):
                    nc.vector.tensor_tensor(out=sq[k], in0=att[k], in1=att[k], op=OP.mult)

                with tc.tile_pool(name="ps_row", bufs=1, space="PSUM") as ps_row:
                    rows = {}
                    for nm in ("sx0", "sx1", "sxx0", "sxx1"):
                        rows[nm] = ps_row.tile([1, 512], F32, tag=nm, name=nm)
                    for ch in range(2):
                        for k in range(KT):
                            nc.tensor.matmul(
                                out=rows[f"sx{ch}"], lhsT=ones_col,
                                rhs=att[k][:, ch * 512:(ch + 1) * 512],
                                start=(k == 0), stop=(k == KT - 1),
                            )
                        for k in range(KT):
                            nc.tensor.matmul(
                                out=rows[f"sxx{ch}"], lhsT=ones_col,
                                rhs=sq[k][:, ch * 512:(ch + 1) * 512],
                                start=(k == 0), stop=(k == KT - 1),
                            )
                    mrow = rowpool.tile([1, N], F32, tag="mrow", name="mrow")
                    t0 = rowpool.tile([1, N], F32, tag="t0", name="t0")
                    for ch in range(2):
                        sl = slice(ch * 512, (ch + 1) * 512)
                        nc.scalar.mul(out=mrow[:, sl], in_=rows[f"sx{ch}"], mul=1.0 / C)
                        nc.scalar.mul(out=f_t[0:1, sl], in_=rows[f"sx{ch}"], mul=-1.0 / C)
                        nc.scalar.mul(out=t0[:, sl], in_=rows[f"sxx{ch}"], mul=1.0 / C)
                    m2 = rowpool.tile([1, N], F32, tag="m2", name="m2")
                    nc.vector.tensor_tensor(out=m2, in0=mrow, in1=mrow, op=OP.mult)
                    varr = rowpool.tile([1, N], F32, tag="varr", name="varr")
                    nc.vector.tensor_tensor(out=varr, in0=t0, in1=m2, op=OP.subtract)
                    stdrow = rowpool.tile([1, N], F32R, tag="stdrow", name="stdrow")
                    nc.scalar.activation(out=stdrow, in_=varr, func=AF.Sqrt,
                                         bias=eps_row, scale=1.0)
                    # DMA (partition-unconstrained) assembles row 1 of f_t
                    nc.sync.dma_start(out=f_t[1:2, :], in_=stdrow)

                with tc.tile_pool(name="ps_out", bufs=2, space="PSUM") as ps_out, \
                     tc.tile_pool(name="ps_cs", bufs=2, space="PSUM") as ps_cs:
                    for n in range(NT):
                        # column stats -> per-token rsqrt(var+eps) scale
                        cs = ps_cs.tile([128, 4], F32, tag="cs", name="cs")
                        for k in range(KT):
                            nc.tensor.matmul(
                                out=cs[:, 0:2], lhsT=att[k][:, n * 128:(n + 1) * 128],
                                rhs=ones2_t, start=(k == 0), stop=(k == KT - 1))
                        for k in range(KT):
                            nc.tensor.matmul(
                                out=cs[:, 2:4], lhsT=sq[k][:, n * 128:(n + 1) * 128],
                                rhs=ones2_t, start=(k == 0), stop=(k == KT - 1))
                        meanc = rowpool.tile([128, 1], F32, tag="meanc", name="meanc")
                        nc.scalar.mul(out=meanc, in_=cs[:, 0:1], mul=1.0 / C)
                        m2c = rowpool.tile([128, 1], F32, tag="m2c", name="m2c")
                        nc.vector.tensor_tensor(out=m2c, in0=meanc, in1=meanc, op=OP.mult)
                        tc0 = rowpool.tile([128, 1], F32, tag="tc0", name="tc0")
                        nc.scalar.mul(out=tc0, in_=cs[:, 2:3], mul=1.0 / C)
                        varc = rowpool.tile([128, 1], F32, tag="varc", name="varc")
                        nc.vector.tensor_tensor(out=varc, in0=tc0, in1=m2c, op=OP.subtract)
                        stdc = rowpool.tile([128, 1], F32, tag="stdc", name="stdc")
                        nc.scalar.activation(out=stdc, in_=varc, func=AF.Sqrt,
                                             bias=eps_col, scale=1.0)
                        nc.vector.reciprocal(out=acol[:, n:n + 1], in_=stdc)

                        po = ps_out.tile([128, C], F32, tag="po", name="po")
                        for c0, cw in ((0, 512), (512, C - 512)):
                            for k in range(KT):
                                nc.tensor.matmul(
                                    out=po[:, c0:c0 + cw],
                                    lhsT=att[k][:, n * 128:(n + 1) * 128],
                                    rhs=wo_t[k][:, c0:c0 + cw],
                                    start=(k == 0), stop=False,
                                )
                            nc.tensor.matmul(
                                out=po[:, c0:c0 + cw],
                                lhsT=f_t[:, n * 128:(n + 1) * 128],
                                rhs=extra_t[:, c0:c0 + cw],
                                start=False, stop=True,
                            )
                        yt = ypool.tile([128, C], F32, tag="yt", name="yt")
                        nc.vector.tensor_scalar(
                            out=yt, in0=po, scalar1=acol[:, n:n + 1], scalar2=None,
                            op0=OP.mult)
                        nc.sync.dma_start(out=y_d[n * 128:(n + 1) * 128, :], in_=yt)

    nc.compile()
    return nc


def prepare_in_maps(x, Wq, bq, Wk, bk, Wv, bv, Wo, bo, ln_g, ln_b):
    x = np.asarray(x, np.float32)
    Wq = np.asarray(Wq, np.float32); bq = np.asarray(bq, np.float32)
    Wk = np.asarray(Wk, np.float32)
    Wv = np.asarray(Wv, np.float32); bv = np.asarray(bv, np.float32)
    Wo = np.asarray(Wo, np.float32); bo = np.asarray(bo, np.float32)
    ln_g = np.asarray(ln_g, np.float32); ln_b = np.asarray(ln_b, np.float32)

    wq = np.ascontiguousarray(Wq.T) * SCALE
    wk = np.ascontiguousarray(Wk.T)
    wv = np.ascontiguousarray(Wv.T)            # [C, C]
    wv_ext = np.zeros((C, VW), np.float32)
    bve = np.zeros((VW,), np.float32)
    for h in range(H):
        wv_ext[:, h * 65: h * 65 + 64] = wv[:, h * 64:(h + 1) * 64]
        bve[h * 65: h * 65 + 64] = bv[h * 64:(h + 1) * 64]
        bve[h * 65 + 64] = 1.0                 # ones column for denominators
    wo = ln_g[:, None] * np.ascontiguousarray(Wo.T)
    bo_eff = bo + ln_b @ Wo.T
    extra = np.stack([wo.sum(axis=0), bo_eff]).astype(np.float32)
    bqs = bq * SCALE
    onesd = np.ones(128, np.float32)

    shared = {"wo": wo, "extra": extra, "bqs": bqs, "bve": bve, "onesd": onesd,
              "ones2": np.ones((128, 2), np.float32)}
    in_maps = []
    for b in range(B):
        xT = np.ascontiguousarray(x[b].T)      # [C, N]
        mega = np.concatenate([wq, wk, wv_ext, xT], axis=1)
        in_maps.append({"mega": mega, **shared})
    return in_maps


_NC_CACHE = []


def _get_nc():
    if not _NC_CACHE:
        _NC_CACHE.append(build_kernel())
    return _NC_CACHE[0]


def kernel(**inputs) -> np.ndarray:
    nc = _get_nc()
    in_maps = prepare_in_maps(**inputs)
    res = run_bass_kernel_spmd(nc, in_maps, core_ids=list(range(B)))
    return np.stack([res.results[b]["y"] for b in range(B)], axis=0)
